# revision 31
# baseline (speedup 1.0000x reference)
"""Trainium2 Bass kernel for nn_IxformerQuantMoe (quantized top-2 MoE, E=8 experts).

Strategy (expert-parallel across 8 NeuronCores):
  - Host computes the fp32 gate (softmax + top-2 + renormalize), routes tokens
    to experts (padded to a common capacity C), and performs the per-token
    dynamic int8 quantization of the routed input, delivering it pre-transposed
    (contraction-major, int8 values exact in bf16).
  - Core e runs the quantized FFN for expert e, software-pipelined two tiles
    deep so the PE array never idles:
      PE stream per iteration: fc1(it) -> fc2(it-1) -> requant-transposes(it).
    The requant chain (dequant+SwiGLU, abs-max, rescale, re-quantize) for tile
    `it` executes on the vector/scalar engines concurrently with fc2(it-1).
    int8 x int8 products are computed exactly on the PE in bf16 (fp32 PSUM).
  - Host scatter-adds each expert's output rows into the final [T, H] output.
"""

import os
import sys

for _p in ("/opt/trn_rl_repo", "/root/.axon_site/_ro/trn_rl_repo"):
    if os.path.isdir(_p) and _p not in sys.path:
        sys.path.insert(0, _p)

import numpy as np
import ml_dtypes

import concourse.bass as bass
import concourse.bacc as bacc
import concourse.tile as tile
from concourse import mybir
from concourse.bass import ds, ts
from concourse.bass_utils import run_bass_kernel_spmd
from concourse.masks import make_identity

T, H, I, E, TOPK = 4096, 2048, 1408, 8, 2
KT1 = H // 128     # 16 k-tiles for fc1 contraction
KT2 = I // 128     # 11 k-tiles for fc2 contraction
TWO23 = 12582912.0  # 1.5*2^23: fp32 add/sub rounds to nearest integer (RNE)

F32 = mybir.dt.float32
BF16 = mybir.dt.bfloat16
MUL = None  # set at import below
_cache = {}
LAST_EXEC_NS = None


def x_like(handle):
    return handle[:]


def _bcast128(handle, off, n):
    """AP reading handle[off:off+n] replicated across 128 partitions."""
    ap = handle[:][ds(off, n)]
    return bass.AP(tensor=ap.tensor, offset=ap.offset, ap=[[0, 128]] + list(ap.ap))


def _build_program(C):
    nt = C // 128
    nc = bacc.Bacc(None, target_bir_lowering=False)
    mul = mybir.AluOpType.mult
    amax = mybir.AluOpType.max
    aadd = mybir.AluOpType.add

    # qt carries the pre-transposed bf16 activations plus, in 4 trailing bf16
    # columns, the bit-pattern of the per-token fp32 (r, s_in) pair — avoids
    # tiny per-partition DMAs, which poison a DMA ring for ~10us each.
    QTW = KT1 * 128 + 4
    qt_d = nc.declare_dram_parameter("qt", [nt, 128, QTW], BF16, isOutput=False)
    w13_d = nc.declare_dram_parameter("w13t", [KT1, 128, 2 * I], BF16, isOutput=False)
    w13i_d = nc.declare_dram_parameter(
        "w13i", [KT1, 128, 2 * I], mybir.dt.int8, isOutput=False
    )
    w2_d = nc.declare_dram_parameter("w2t", [KT2, 128, H], BF16, isOutput=False)
    s13_d = nc.declare_dram_parameter("s13", [2 * I], F32, isOutput=False)
    s2w_d = nc.declare_dram_parameter("s2w", [H], F32, isOutput=False)
    y_d = nc.declare_dram_parameter("y", [C, H], BF16, isOutput=True)

    fc1_groups = [(0, 512), (512, 512), (1024, 384)]

    with tile.TileContext(nc) as tc:
        with (
            tc.tile_pool(name="singles", bufs=1) as singles,
            tc.tile_pool(name="qtp", bufs=3) as qtp,
            tc.tile_pool(name="gp", bufs=2) as gp,
            tc.tile_pool(name="up", bufs=2) as up,
            tc.tile_pool(name="actp", bufs=1) as actp,
            tc.tile_pool(name="tp", bufs=2) as tp,
            tc.tile_pool(name="qasp", bufs=1) as qasp,
            tc.tile_pool(name="qaktp", bufs=1) as qaktp,
            tc.tile_pool(name="ycp", bufs=3) as ycp,
            tc.tile_pool(name="sp", bufs=2) as sp,
            tc.tile_pool(name="ps1", bufs=2, space="PSUM") as ps1,
            tc.tile_pool(name="ps2", bufs=1, space="PSUM") as ps2,
            tc.tile_pool(name="pst", bufs=1, space="PSUM") as pst,
        ):
            # ---- resident weights / scales; w13 split over 4 DMA queues ----
            w13_sb = singles.tile([128, KT1, 2 * I], BF16)
            w2_sb = singles.tile([128, KT2, H], BF16)
            s13g_b = singles.tile([128, I], F32)
            s13u_b = singles.tile([128, I], F32)
            s2w_b = singles.tile([128, H], F32)
            ident = singles.tile([128, 128], BF16)
            zero_b = singles.tile([128, 1], F32)

            qt_tiles = [None] * nt

            def load_tile_inputs(it, eng=None):
                eng = eng or nc.sync
                q = qtp.tile([128, QTW], BF16, name=f"qtt{it}", tag="qt")
                eng.dma_start(q, x_like(qt_d)[it])
                qt_tiles[it] = q

            # Lead-in schedule: all DMA rings share one pool of 16 DMA
            # engines (~320GB/s aggregate), so the w13 stream is byte-bound.
            # qt(0) heads the sync ring so the first matmul isn't queued
            # behind it on the ring that carries w13 k=0; gpsimd's share of
            # w13 rides as int8 with an SWDGE casting DMA (half the read
            # bytes); scale broadcasts go mid-stream; w2 trails on gpsimd.
            load_tile_inputs(0, nc.sync)
            make_identity(nc, ident)
            nc.vector.memset(zero_b, 0.0)
            # all of w13 rides the gpsimd SWDGE as int8 casting DMAs (half
            # the read bytes); w2 takes the scalar HW ring as bf16; the sync
            # ring carries activations and the scale broadcasts.
            for k in range(KT1):
                nc.gpsimd.dma_start(w13_sb[:, k, :], x_like(w13i_d)[k])
                if k == 7:
                    nc.sync.dma_start(s13g_b, _bcast128(s13_d, 0, I))
                    nc.sync.dma_start(s13u_b, _bcast128(s13_d, I, I))
            load_tile_inputs(1, nc.sync)
            nc.sync.dma_start(s2w_b, _bcast128(s2w_d, 0, H))
            for k in range(KT2):
                nc.scalar.dma_start(w2_sb[:, k, :], x_like(w2_d)[k])

            qa_kt_tiles = [None] * nt
            ct_tiles = [None] * nt

            def fc2_y(jt):
                qa_kt = qa_kt_tiles[jt]
                c_t = ct_tiles[jt]
                for op_ in range(2):
                    pa = ps2.tile([128, 512], F32, tag="pa")
                    pb = ps2.tile([128, 512], F32, tag="pb")
                    for k in range(KT2):
                        nc.tensor.matmul(
                            pa, qa_kt[:, ts(k, 128)], w2_sb[:, k, ts(2 * op_, 512)],
                            start=(k == 0), stop=(k == KT2 - 1),
                        )
                        nc.tensor.matmul(
                            pb, qa_kt[:, ts(k, 128)], w2_sb[:, k, ts(2 * op_ + 1, 512)],
                            start=(k == 0), stop=(k == KT2 - 1),
                        )
                    for which, pp in ((0, pa), (1, pb)):
                        oc = (2 * op_ + which) * 512
                        yc = ycp.tile([128, 512], BF16)
                        nc.vector.scalar_tensor_tensor(
                            yc, pp, c_t, s2w_b[:, ds(oc, 512)], mul, mul
                        )
                        qeng = (nc.sync, nc.scalar, nc.gpsimd, nc.scalar)[2 * op_ + which]
                        qeng.dma_start(
                            x_like(y_d)[jt * 128 : (jt + 1) * 128, ds(oc, 512)], yc
                        )

            for it in range(nt):
                if it + 2 < nt:
                    load_tile_inputs(it + 2)
                qt_t = qt_tiles[it]
                rs_ap = qt_t[:, KT1 * 128 : KT1 * 128 + 4].bitcast(F32)
                r_ap = rs_ap[:, 0:1]
                sin_ap = rs_ap[:, 1:2]

                # ---- fc1 + per-group dequant/SwiGLU + group abs-max ----
                act_t = actp.tile([128, I], F32)
                mgs = []
                for gi, (off, cw) in enumerate(fc1_groups):
                    pg = ps1.tile([128, 512], F32, tag="pg")
                    pu = ps1.tile([128, 512], F32, tag="pu")
                    for k in range(KT1):
                        nc.tensor.matmul(
                            pg[:, :cw], qt_t[:, ts(k, 128)], w13_sb[:, k, ds(off, cw)],
                            start=(k == 0), stop=(k == KT1 - 1),
                        )
                        nc.tensor.matmul(
                            pu[:, :cw], qt_t[:, ts(k, 128)], w13_sb[:, k, ds(I + off, cw)],
                            start=(k == 0), stop=(k == KT1 - 1),
                        )
                    # act' = silu(g*s13g*s_in) * (u*s13u); true act = act' * s_in
                    g_sc = gp.tile([128, 512], F32)
                    nc.vector.scalar_tensor_tensor(
                        g_sc[:, :cw], pg[:, :cw], sin_ap, s13g_b[:, ds(off, cw)],
                        mul, mul,
                    )
                    nc.scalar.activation(
                        g_sc[:, :cw], g_sc[:, :cw],
                        mybir.ActivationFunctionType.Silu, bias=zero_b,
                    )
                    u_sc = up.tile([128, 512], F32)
                    nc.vector.tensor_tensor(
                        u_sc[:, :cw], pu[:, :cw], s13u_b[:, ds(off, cw)], mul
                    )
                    nc.vector.tensor_tensor(
                        act_t[:, ds(off, cw)], g_sc[:, :cw], u_sc[:, :cw], mul
                    )
                    mg = sp.tile([128, 1], F32, tag=f"mg{gi}")
                    nc.vector.tensor_reduce(
                        mg, act_t[:, ds(off, cw)], axis=mybir.AxisListType.X,
                        op=amax, apply_absolute_value=True,
                    )
                    mgs.append(mg)

                # ---- requant scale chain (vector; overlaps fc2(it-1) on PE) ----
                m2 = sp.tile([128, 1], F32, tag="m2")
                nc.vector.tensor_tensor(m2, mgs[0], mgs[1], amax)
                m2b = sp.tile([128, 1], F32, tag="m2b")
                nc.vector.tensor_tensor(m2b, m2, mgs[2], amax)
                mt2 = sp.tile([128, 1], F32, tag="mt2")
                nc.vector.tensor_tensor(mt2, m2b, sin_ap, mul)
                s_tr = sp.tile([128, 1], F32, tag="s_tr")
                nc.vector.tensor_scalar(s_tr, mt2, 1.0 / 127.0, 1e-8, mul, amax)
                inv_tr = sp.tile([128, 1], F32, tag="inv_tr")
                nc.vector.reciprocal(inv_tr, s_tr)
                sc_eff = sp.tile([128, 1], F32, tag="sc_eff")  # s_in / s_true
                nc.vector.tensor_tensor(sc_eff, sin_ap, inv_tr, mul)
                c_t = sp.tile([128, 1], F32, tag="c")  # r * s_true
                nc.vector.tensor_tensor(c_t, r_ap, s_tr, mul)
                ct_tiles[it] = c_t[:, 0:1]

                # ---- re-quantize act (round via +/- 1.5*2^23, RNE) ----
                # chunked so the first transposes can start before the tail
                # chunks convert (the PE meanwhile runs fc2(it-1))
                qa_sb = qasp.tile([128, I], BF16)
                for off, cw in fc1_groups:
                    t_c = tp.tile([128, 512], F32)
                    nc.vector.tensor_scalar(
                        t_c[:, :cw], act_t[:, ds(off, cw)], sc_eff[:, 0:1], TWO23,
                        mul, aadd,
                    )
                    nc.vector.tensor_scalar(
                        qa_sb[:, ds(off, cw)], t_c[:, :cw], -TWO23, None, aadd
                    )

                # ---- fc2 + combine for the PREVIOUS tile (keeps PE busy) ----
                if it > 0:
                    fc2_y(it - 1)

                # ---- transpose re-quantized act to contraction-major ----
                pst_t = pst.tile([128, KT2 * 128], BF16)
                for k in range(KT2):
                    nc.tensor.transpose(
                        pst_t[:, ts(k, 128)], qa_sb[:, ts(k, 128)], ident
                    )
                qa_kt = qaktp.tile([128, KT2 * 128], BF16)
                nc.scalar.copy(qa_kt, pst_t)
                qa_kt_tiles[it] = qa_kt

            fc2_y(nt - 1)

    nc.finalize()
    return nc


def kernel(hidden_states, gate_weight, w13_weight, w13_weight_scale,
           w2_weight, w2_weight_scale):
    x = np.ascontiguousarray(np.asarray(hidden_states, dtype=np.float32))
    gw = np.asarray(gate_weight, dtype=np.float32)
    w13 = np.asarray(w13_weight)
    s13 = np.ascontiguousarray(np.asarray(w13_weight_scale, dtype=np.float32))
    w2 = np.asarray(w2_weight)
    s2w = np.ascontiguousarray(np.asarray(w2_weight_scale, dtype=np.float32))

    # ---- host routing: fp32 gate, softmax, top-2, renormalize ----
    logits = (x @ gw.T).astype(np.float32)
    p = np.exp(logits - logits.max(axis=1, keepdims=True), dtype=np.float32)
    p = (p / p.sum(axis=1, keepdims=True)).astype(np.float32)
    topi = np.argsort(-p, axis=1, kind="stable")[:, :TOPK]  # ties -> lower index
    topv = np.take_along_axis(p, topi, axis=1).astype(np.float32)
    gates = (topv / topv.sum(axis=1, keepdims=True)).astype(np.float32)

    # ---- host dynamic per-token int8 quant (mirrors reference fp32 math) ----
    ax = np.abs(x).max(axis=1, keepdims=True).astype(np.float32)
    s_in = np.maximum(ax / np.float32(127.0), np.float32(1e-8)).astype(np.float32)
    q = np.clip(np.round(x / s_in), -127.0, 127.0).astype(np.float32)
    q_bf = q.astype(ml_dtypes.bfloat16)  # int8 values are exact in bf16

    idxs, rvals = [], []
    for e in range(E):
        sel = topi == e
        tok = np.nonzero(sel.any(axis=1))[0]
        r = (gates * sel)[tok].sum(axis=1).astype(np.float32)
        idxs.append(tok)
        rvals.append(r)

    cap = max(256, max(len(t) for t in idxs))
    C = ((cap + 127) // 128) * 128
    nt = C // 128

    if C not in _cache:
        _cache[C] = _build_program(C)
    nc = _cache[C]

    in_maps = []
    for e in range(E):
        n_e = len(idxs[e])
        qg = np.zeros((C, H), dtype=ml_dtypes.bfloat16)
        qg[:n_e] = q_bf[idxs[e]]
        rs = np.zeros((C, 2), dtype=np.float32)
        rs[:n_e, 0] = rvals[e]
        rs[:n_e, 1] = s_in[idxs[e], 0]
        # [tile, 128 h-in-k, k, 128 tok] blocks + bit-punned (r, s_in) tail
        qt_host = np.empty((nt, 128, KT1 * 128 + 4), dtype=ml_dtypes.bfloat16)
        qt_host[:, :, : KT1 * 128] = (
            qg.reshape(nt, 128, KT1, 128).transpose(0, 3, 2, 1)
            .reshape(nt, 128, KT1 * 128)
        )
        qt_host[:, :, KT1 * 128 :] = (
            rs.reshape(nt, 128, 2).view(np.uint16).view(ml_dtypes.bfloat16)
        )
        w13t = np.ascontiguousarray(w13[e].T).reshape(KT1, 128, 2 * I)
        w2t = np.ascontiguousarray(w2[e].T).reshape(KT2, 128, H)
        in_maps.append({
            "qt": qt_host,
            "w13t": w13t.astype(ml_dtypes.bfloat16),
            "w13i": w13t.astype(np.int8),
            "w2t": w2t.astype(ml_dtypes.bfloat16),
            "s13": np.ascontiguousarray(s13[e]),
            "s2w": np.ascontiguousarray(s2w[e]),
        })

    trace = bool(int(os.environ.get("MOE_TRACE", "0")))
    br = run_bass_kernel_spmd(nc, in_maps, list(range(E)), trace=trace)
    global LAST_EXEC_NS
    LAST_EXEC_NS = br.exec_time_ns
    res = br.results

    out = np.zeros((T, H), dtype=np.float32)
    for e in range(E):
        n_e = len(idxs[e])
        if n_e:
            out[idxs[e]] += np.asarray(res[e]["y"])[:n_e].astype(np.float32)
    return out


# revision 32
# speedup vs baseline: 1.0029x; 1.0029x over previous
"""Trainium2 Bass kernel for nn_IxformerQuantMoe (quantized top-2 MoE, E=8 experts).

Strategy (expert-parallel across 8 NeuronCores):
  - Host computes the fp32 gate (softmax + top-2 + renormalize), routes tokens
    to experts (padded to a common capacity C), and performs the per-token
    dynamic int8 quantization of the routed input, delivering it pre-transposed
    (contraction-major, int8 values exact in bf16).
  - Core e runs the quantized FFN for expert e, software-pipelined two tiles
    deep so the PE array never idles:
      PE stream per iteration: fc1(it) -> fc2(it-1) -> requant-transposes(it).
    The requant chain (dequant+SwiGLU, abs-max, rescale, re-quantize) for tile
    `it` executes on the vector/scalar engines concurrently with fc2(it-1).
    int8 x int8 products are computed exactly on the PE in bf16 (fp32 PSUM).
  - Host scatter-adds each expert's output rows into the final [T, H] output.
"""

import os
import sys

for _p in ("/opt/trn_rl_repo", "/root/.axon_site/_ro/trn_rl_repo"):
    if os.path.isdir(_p) and _p not in sys.path:
        sys.path.insert(0, _p)

import numpy as np
import ml_dtypes

import concourse.bass as bass
import concourse.bacc as bacc
import concourse.tile as tile
from concourse import mybir
from concourse.bass import ds, ts
from concourse.bass_utils import run_bass_kernel_spmd
from concourse.masks import make_identity

T, H, I, E, TOPK = 4096, 2048, 1408, 8, 2
KT1 = H // 128     # 16 k-tiles for fc1 contraction
KT2 = I // 128     # 11 k-tiles for fc2 contraction
TWO23 = 12582912.0  # 1.5*2^23: fp32 add/sub rounds to nearest integer (RNE)

F32 = mybir.dt.float32
BF16 = mybir.dt.bfloat16
MUL = None  # set at import below
_cache = {}
LAST_EXEC_NS = None


def x_like(handle):
    return handle[:]


def _bcast128(handle, off, n):
    """AP reading handle[off:off+n] replicated across 128 partitions."""
    ap = handle[:][ds(off, n)]
    return bass.AP(tensor=ap.tensor, offset=ap.offset, ap=[[0, 128]] + list(ap.ap))


def _build_program(C):
    nt = C // 128
    nc = bacc.Bacc(None, target_bir_lowering=False)
    mul = mybir.AluOpType.mult
    amax = mybir.AluOpType.max
    aadd = mybir.AluOpType.add

    # qt carries the pre-transposed bf16 activations plus, in 4 trailing bf16
    # columns, the bit-pattern of the per-token fp32 (r, s_in) pair — avoids
    # tiny per-partition DMAs, which poison a DMA ring for ~10us each.
    QTW = KT1 * 128 + 4
    qt_d = nc.declare_dram_parameter("qt", [nt, 128, QTW], BF16, isOutput=False)
    w13_d = nc.declare_dram_parameter("w13t", [KT1, 128, 2 * I], BF16, isOutput=False)
    w13i_d = nc.declare_dram_parameter(
        "w13i", [KT1, 128, 2 * I], mybir.dt.int8, isOutput=False
    )
    w2_d = nc.declare_dram_parameter("w2t", [KT2, 128, H], BF16, isOutput=False)
    s13_d = nc.declare_dram_parameter("s13", [2 * I], F32, isOutput=False)
    s2w_d = nc.declare_dram_parameter("s2w", [H], F32, isOutput=False)
    y_d = nc.declare_dram_parameter("y", [C, H], BF16, isOutput=True)

    fc1_groups = [(0, 512), (512, 512), (1024, 384)]

    with tile.TileContext(nc) as tc:
        with (
            tc.tile_pool(name="singles", bufs=1) as singles,
            tc.tile_pool(name="qtp", bufs=3) as qtp,
            tc.tile_pool(name="gp", bufs=2) as gp,
            tc.tile_pool(name="up", bufs=2) as up,
            tc.tile_pool(name="actp", bufs=1) as actp,
            tc.tile_pool(name="tp", bufs=2) as tp,
            tc.tile_pool(name="qasp", bufs=1) as qasp,
            tc.tile_pool(name="qaktp", bufs=1) as qaktp,
            tc.tile_pool(name="ycp", bufs=3) as ycp,
            tc.tile_pool(name="sp", bufs=2) as sp,
            tc.tile_pool(name="ps1", bufs=2, space="PSUM") as ps1,
            tc.tile_pool(name="ps2", bufs=1, space="PSUM") as ps2,
            tc.tile_pool(name="pst", bufs=1, space="PSUM") as pst,
        ):
            # ---- resident weights / scales; w13 split over 4 DMA queues ----
            w13_sb = singles.tile([128, KT1, 2 * I], BF16)
            w2_sb = singles.tile([128, KT2, H], BF16)
            s13g_b = singles.tile([128, I], F32)
            s13u_b = singles.tile([128, I], F32)
            s2w_b = singles.tile([128, H], F32)
            ident = singles.tile([128, 128], BF16)
            zero_b = singles.tile([128, 1], F32)

            qt_tiles = [None] * nt

            def load_tile_inputs(it, eng=None):
                eng = eng or nc.sync
                q = qtp.tile([128, QTW], BF16, name=f"qtt{it}", tag="qt")
                eng.dma_start(q, x_like(qt_d)[it])
                qt_tiles[it] = q

            # Lead-in schedule: all DMA rings share one pool of 16 DMA
            # engines (~320GB/s aggregate), so the w13 stream is byte-bound.
            # qt(0) heads the sync ring so the first matmul isn't queued
            # behind it on the ring that carries w13 k=0; gpsimd's share of
            # w13 rides as int8 with an SWDGE casting DMA (half the read
            # bytes); scale broadcasts go mid-stream; w2 trails on gpsimd.
            load_tile_inputs(0, nc.sync)
            make_identity(nc, ident)
            nc.vector.memset(zero_b, 0.0)
            # odd w13 k-chunks ride the gpsimd SWDGE as int8 casting DMAs
            # (half the read bytes); even chunks alternate scalar/sync as
            # bf16; w2 trails on scalar; broadcasts mid-stream on sync.
            for k in range(KT1):
                if k % 2 == 1:
                    nc.gpsimd.dma_start(w13_sb[:, k, :], x_like(w13i_d)[k])
                elif k % 4 == 0:
                    nc.scalar.dma_start(w13_sb[:, k, :], x_like(w13_d)[k])
                else:
                    nc.sync.dma_start(w13_sb[:, k, :], x_like(w13_d)[k])
                if k == 11:
                    nc.sync.dma_start(s13g_b, _bcast128(s13_d, 0, I))
                    nc.sync.dma_start(s13u_b, _bcast128(s13_d, I, I))
                    nc.gpsimd.dma_start(s2w_b, _bcast128(s2w_d, 0, H))
            load_tile_inputs(1, nc.sync)
            for k in range(KT2):
                nc.scalar.dma_start(w2_sb[:, k, :], x_like(w2_d)[k])

            qa_kt_tiles = [None] * nt
            ct_tiles = [None] * nt

            def fc2_y(jt):
                qa_kt = qa_kt_tiles[jt]
                c_t = ct_tiles[jt]
                for op_ in range(2):
                    pa = ps2.tile([128, 512], F32, tag="pa")
                    pb = ps2.tile([128, 512], F32, tag="pb")
                    for k in range(KT2):
                        nc.tensor.matmul(
                            pa, qa_kt[:, ts(k, 128)], w2_sb[:, k, ts(2 * op_, 512)],
                            start=(k == 0), stop=(k == KT2 - 1),
                        )
                        nc.tensor.matmul(
                            pb, qa_kt[:, ts(k, 128)], w2_sb[:, k, ts(2 * op_ + 1, 512)],
                            start=(k == 0), stop=(k == KT2 - 1),
                        )
                    for which, pp in ((0, pa), (1, pb)):
                        oc = (2 * op_ + which) * 512
                        yc = ycp.tile([128, 512], BF16)
                        nc.vector.scalar_tensor_tensor(
                            yc, pp, c_t, s2w_b[:, ds(oc, 512)], mul, mul
                        )
                        qeng = (nc.sync, nc.scalar, nc.gpsimd, nc.scalar)[2 * op_ + which]
                        qeng.dma_start(
                            x_like(y_d)[jt * 128 : (jt + 1) * 128, ds(oc, 512)], yc
                        )

            for it in range(nt):
                if it + 2 < nt:
                    load_tile_inputs(it + 2)
                qt_t = qt_tiles[it]
                rs_ap = qt_t[:, KT1 * 128 : KT1 * 128 + 4].bitcast(F32)
                r_ap = rs_ap[:, 0:1]
                sin_ap = rs_ap[:, 1:2]

                # ---- fc1 + per-group dequant/SwiGLU + group abs-max ----
                act_t = actp.tile([128, I], F32)
                mgs = []
                for gi, (off, cw) in enumerate(fc1_groups):
                    pg = ps1.tile([128, 512], F32, tag="pg")
                    pu = ps1.tile([128, 512], F32, tag="pu")
                    for k in range(KT1):
                        nc.tensor.matmul(
                            pg[:, :cw], qt_t[:, ts(k, 128)], w13_sb[:, k, ds(off, cw)],
                            start=(k == 0), stop=(k == KT1 - 1),
                        )
                        nc.tensor.matmul(
                            pu[:, :cw], qt_t[:, ts(k, 128)], w13_sb[:, k, ds(I + off, cw)],
                            start=(k == 0), stop=(k == KT1 - 1),
                        )
                    # act' = silu(g*s13g*s_in) * (u*s13u); true act = act' * s_in
                    g_sc = gp.tile([128, 512], F32)
                    nc.vector.scalar_tensor_tensor(
                        g_sc[:, :cw], pg[:, :cw], sin_ap, s13g_b[:, ds(off, cw)],
                        mul, mul,
                    )
                    nc.scalar.activation(
                        g_sc[:, :cw], g_sc[:, :cw],
                        mybir.ActivationFunctionType.Silu, bias=zero_b,
                    )
                    u_sc = up.tile([128, 512], F32)
                    nc.vector.tensor_tensor(
                        u_sc[:, :cw], pu[:, :cw], s13u_b[:, ds(off, cw)], mul
                    )
                    nc.vector.tensor_tensor(
                        act_t[:, ds(off, cw)], g_sc[:, :cw], u_sc[:, :cw], mul
                    )
                    mg = sp.tile([128, 1], F32, tag=f"mg{gi}")
                    nc.vector.tensor_reduce(
                        mg, act_t[:, ds(off, cw)], axis=mybir.AxisListType.X,
                        op=amax, apply_absolute_value=True,
                    )
                    mgs.append(mg)

                # ---- requant scale chain (vector; overlaps fc2(it-1) on PE) ----
                m2 = sp.tile([128, 1], F32, tag="m2")
                nc.vector.tensor_tensor(m2, mgs[0], mgs[1], amax)
                m2b = sp.tile([128, 1], F32, tag="m2b")
                nc.vector.tensor_tensor(m2b, m2, mgs[2], amax)
                mt2 = sp.tile([128, 1], F32, tag="mt2")
                nc.vector.tensor_tensor(mt2, m2b, sin_ap, mul)
                s_tr = sp.tile([128, 1], F32, tag="s_tr")
                nc.vector.tensor_scalar(s_tr, mt2, 1.0 / 127.0, 1e-8, mul, amax)
                inv_tr = sp.tile([128, 1], F32, tag="inv_tr")
                nc.vector.reciprocal(inv_tr, s_tr)
                sc_eff = sp.tile([128, 1], F32, tag="sc_eff")  # s_in / s_true
                nc.vector.tensor_tensor(sc_eff, sin_ap, inv_tr, mul)
                c_t = sp.tile([128, 1], F32, tag="c")  # r * s_true
                nc.vector.tensor_tensor(c_t, r_ap, s_tr, mul)
                ct_tiles[it] = c_t[:, 0:1]

                # ---- re-quantize act (round via +/- 1.5*2^23, RNE) ----
                # chunked so the first transposes can start before the tail
                # chunks convert (the PE meanwhile runs fc2(it-1))
                qa_sb = qasp.tile([128, I], BF16)
                for off, cw in fc1_groups:
                    t_c = tp.tile([128, 512], F32)
                    nc.vector.tensor_scalar(
                        t_c[:, :cw], act_t[:, ds(off, cw)], sc_eff[:, 0:1], TWO23,
                        mul, aadd,
                    )
                    nc.vector.tensor_scalar(
                        qa_sb[:, ds(off, cw)], t_c[:, :cw], -TWO23, None, aadd
                    )

                # ---- fc2 + combine for the PREVIOUS tile (keeps PE busy) ----
                if it > 0:
                    fc2_y(it - 1)

                # ---- transpose re-quantized act to contraction-major ----
                pst_t = pst.tile([128, KT2 * 128], BF16)
                for k in range(KT2):
                    nc.tensor.transpose(
                        pst_t[:, ts(k, 128)], qa_sb[:, ts(k, 128)], ident
                    )
                qa_kt = qaktp.tile([128, KT2 * 128], BF16)
                nc.scalar.copy(qa_kt, pst_t)
                qa_kt_tiles[it] = qa_kt

            fc2_y(nt - 1)

    nc.finalize()
    return nc


def kernel(hidden_states, gate_weight, w13_weight, w13_weight_scale,
           w2_weight, w2_weight_scale):
    x = np.ascontiguousarray(np.asarray(hidden_states, dtype=np.float32))
    gw = np.asarray(gate_weight, dtype=np.float32)
    w13 = np.asarray(w13_weight)
    s13 = np.ascontiguousarray(np.asarray(w13_weight_scale, dtype=np.float32))
    w2 = np.asarray(w2_weight)
    s2w = np.ascontiguousarray(np.asarray(w2_weight_scale, dtype=np.float32))

    # ---- host routing: fp32 gate, softmax, top-2, renormalize ----
    logits = (x @ gw.T).astype(np.float32)
    p = np.exp(logits - logits.max(axis=1, keepdims=True), dtype=np.float32)
    p = (p / p.sum(axis=1, keepdims=True)).astype(np.float32)
    topi = np.argsort(-p, axis=1, kind="stable")[:, :TOPK]  # ties -> lower index
    topv = np.take_along_axis(p, topi, axis=1).astype(np.float32)
    gates = (topv / topv.sum(axis=1, keepdims=True)).astype(np.float32)

    # ---- host dynamic per-token int8 quant (mirrors reference fp32 math) ----
    ax = np.abs(x).max(axis=1, keepdims=True).astype(np.float32)
    s_in = np.maximum(ax / np.float32(127.0), np.float32(1e-8)).astype(np.float32)
    q = np.clip(np.round(x / s_in), -127.0, 127.0).astype(np.float32)
    q_bf = q.astype(ml_dtypes.bfloat16)  # int8 values are exact in bf16

    idxs, rvals = [], []
    for e in range(E):
        sel = topi == e
        tok = np.nonzero(sel.any(axis=1))[0]
        r = (gates * sel)[tok].sum(axis=1).astype(np.float32)
        idxs.append(tok)
        rvals.append(r)

    cap = max(256, max(len(t) for t in idxs))
    C = ((cap + 127) // 128) * 128
    nt = C // 128

    if C not in _cache:
        _cache[C] = _build_program(C)
    nc = _cache[C]

    in_maps = []
    for e in range(E):
        n_e = len(idxs[e])
        qg = np.zeros((C, H), dtype=ml_dtypes.bfloat16)
        qg[:n_e] = q_bf[idxs[e]]
        rs = np.zeros((C, 2), dtype=np.float32)
        rs[:n_e, 0] = rvals[e]
        rs[:n_e, 1] = s_in[idxs[e], 0]
        # [tile, 128 h-in-k, k, 128 tok] blocks + bit-punned (r, s_in) tail
        qt_host = np.empty((nt, 128, KT1 * 128 + 4), dtype=ml_dtypes.bfloat16)
        qt_host[:, :, : KT1 * 128] = (
            qg.reshape(nt, 128, KT1, 128).transpose(0, 3, 2, 1)
            .reshape(nt, 128, KT1 * 128)
        )
        qt_host[:, :, KT1 * 128 :] = (
            rs.reshape(nt, 128, 2).view(np.uint16).view(ml_dtypes.bfloat16)
        )
        w13t = np.ascontiguousarray(w13[e].T).reshape(KT1, 128, 2 * I)
        w2t = np.ascontiguousarray(w2[e].T).reshape(KT2, 128, H)
        in_maps.append({
            "qt": qt_host,
            "w13t": w13t.astype(ml_dtypes.bfloat16),
            "w13i": w13t.astype(np.int8),
            "w2t": w2t.astype(ml_dtypes.bfloat16),
            "s13": np.ascontiguousarray(s13[e]),
            "s2w": np.ascontiguousarray(s2w[e]),
        })

    trace = bool(int(os.environ.get("MOE_TRACE", "0")))
    br = run_bass_kernel_spmd(nc, in_maps, list(range(E)), trace=trace)
    global LAST_EXEC_NS
    LAST_EXEC_NS = br.exec_time_ns
    res = br.results

    out = np.zeros((T, H), dtype=np.float32)
    for e in range(E):
        n_e = len(idxs[e])
        if n_e:
            out[idxs[e]] += np.asarray(res[e]["y"])[:n_e].astype(np.float32)
    return out


# revision 34
# speedup vs baseline: 1.0415x; 1.0385x over previous
"""Trainium2 Bass kernel for nn_IxformerQuantMoe (quantized top-2 MoE, E=8 experts).

Strategy (expert-parallel across 8 NeuronCores):
  - Host computes the fp32 gate (softmax + top-2 + renormalize), routes tokens
    to experts (padded to a common capacity C), and performs the per-token
    dynamic int8 quantization of the routed input, delivering it pre-transposed
    (contraction-major, int8 values exact in bf16).
  - Core e runs the quantized FFN for expert e, software-pipelined two tiles
    deep so the PE array never idles:
      PE stream per iteration: fc1(it) -> fc2(it-1) -> requant-transposes(it).
    The requant chain (dequant+SwiGLU, abs-max, rescale, re-quantize) for tile
    `it` executes on the vector/scalar engines concurrently with fc2(it-1).
    int8 x int8 products are computed exactly on the PE in bf16 (fp32 PSUM).
  - Host scatter-adds each expert's output rows into the final [T, H] output.
"""

import os
import sys

for _p in ("/opt/trn_rl_repo", "/root/.axon_site/_ro/trn_rl_repo"):
    if os.path.isdir(_p) and _p not in sys.path:
        sys.path.insert(0, _p)

import numpy as np
import ml_dtypes

import concourse.bass as bass
import concourse.bacc as bacc
import concourse.tile as tile
from concourse import mybir
from concourse.bass import ds, ts
from concourse.bass_utils import run_bass_kernel_spmd
from concourse.masks import make_identity

T, H, I, E, TOPK = 4096, 2048, 1408, 8, 2
KT1 = H // 128     # 16 k-tiles for fc1 contraction
KT2 = I // 128     # 11 k-tiles for fc2 contraction
TWO23 = 12582912.0  # 1.5*2^23: fp32 add/sub rounds to nearest integer (RNE)

F32 = mybir.dt.float32
BF16 = mybir.dt.bfloat16
MUL = None  # set at import below
_cache = {}
LAST_EXEC_NS = None


def x_like(handle):
    return handle[:]


def _bcast128(handle, off, n):
    """AP reading handle[off:off+n] replicated across 128 partitions."""
    ap = handle[:][ds(off, n)]
    return bass.AP(tensor=ap.tensor, offset=ap.offset, ap=[[0, 128]] + list(ap.ap))


def _build_program(C):
    nt = C // 128
    nc = bacc.Bacc(None, target_bir_lowering=False)
    mul = mybir.AluOpType.mult
    amax = mybir.AluOpType.max
    aadd = mybir.AluOpType.add

    # qt carries the pre-transposed bf16 activations plus, in 4 trailing bf16
    # columns, the bit-pattern of the per-token fp32 (r, s_in) pair — avoids
    # tiny per-partition DMAs, which poison a DMA ring for ~10us each.
    QTW = KT1 * 128 + 4
    qt_d = nc.declare_dram_parameter("qt", [nt, 128, QTW], BF16, isOutput=False)
    w13_d = nc.declare_dram_parameter("w13t", [KT1, 128, 2 * I], BF16, isOutput=False)
    w13i_d = nc.declare_dram_parameter(
        "w13i", [KT1, 128, 2 * I], mybir.dt.int8, isOutput=False
    )
    w2_d = nc.declare_dram_parameter("w2t", [KT2, 128, H], BF16, isOutput=False)
    s13_d = nc.declare_dram_parameter("s13", [2 * I], F32, isOutput=False)
    s2w_d = nc.declare_dram_parameter("s2w", [H], F32, isOutput=False)
    y_d = nc.declare_dram_parameter("y", [C, H], BF16, isOutput=True)

    fc1_groups = [(0, 512), (512, 512), (1024, 384)]

    with tile.TileContext(nc) as tc:
        with (
            tc.tile_pool(name="singles", bufs=1) as singles,
            tc.tile_pool(name="qtp", bufs=3) as qtp,
            tc.tile_pool(name="gp", bufs=2) as gp,
            tc.tile_pool(name="up", bufs=2) as up,
            tc.tile_pool(name="actp", bufs=1) as actp,
            tc.tile_pool(name="tp", bufs=2) as tp,
            tc.tile_pool(name="qasp", bufs=1) as qasp,
            tc.tile_pool(name="qaktp", bufs=1) as qaktp,
            tc.tile_pool(name="ycp", bufs=3) as ycp,
            tc.tile_pool(name="sp", bufs=2) as sp,
            tc.tile_pool(name="ps1", bufs=2, space="PSUM") as ps1,
            tc.tile_pool(name="ps2", bufs=1, space="PSUM") as ps2,
            tc.tile_pool(name="pst", bufs=1, space="PSUM") as pst,
        ):
            # ---- resident weights / scales; w13 split over 4 DMA queues ----
            w13_sb = singles.tile([128, KT1, 2 * I], BF16)
            w2_sb = singles.tile([128, KT2, H], BF16)
            s13g_b = singles.tile([128, I], F32)
            s13u_b = singles.tile([128, I], F32)
            s2w_b = singles.tile([128, H], F32)
            ident = singles.tile([128, 128], BF16)
            zero_b = singles.tile([128, 1], F32)

            qt_tiles = [None] * nt

            def load_tile_inputs(it, eng=None):
                eng = eng or nc.sync
                q = qtp.tile([128, QTW], BF16, name=f"qtt{it}", tag="qt")
                eng.dma_start(q, x_like(qt_d)[it])
                qt_tiles[it] = q

            # Lead-in schedule: all DMA rings share one pool of 16 DMA
            # engines (~320GB/s aggregate), so the w13 stream is byte-bound.
            # qt(0) heads the sync ring so the first matmul isn't queued
            # behind it on the ring that carries w13 k=0; gpsimd's share of
            # w13 rides as int8 with an SWDGE casting DMA (half the read
            # bytes); scale broadcasts go mid-stream; w2 trails on gpsimd.
            load_tile_inputs(0, nc.sync)
            make_identity(nc, ident)
            nc.vector.memset(zero_b, 0.0)
            # every third w13 k-chunk rides the gpsimd SWDGE as an int8
            # casting DMA (half the read bytes; more than that overloads the
            # SWDGE); the rest alternate scalar/sync as bf16; broadcasts
            # mid-stream; w2 trails on gpsimd.
            rings = (nc.scalar, nc.gpsimd, nc.sync)
            for k in range(KT1):
                if k % 3 == 1:
                    nc.gpsimd.dma_start(w13_sb[:, k, :], x_like(w13i_d)[k])
                else:
                    rings[k % 3].dma_start(w13_sb[:, k, :], x_like(w13_d)[k])
                if k == 11:
                    nc.sync.dma_start(s13g_b, _bcast128(s13_d, 0, I))
                    nc.scalar.dma_start(s13u_b, _bcast128(s13_d, I, I))
                    nc.gpsimd.dma_start(s2w_b, _bcast128(s2w_d, 0, H))
            load_tile_inputs(1, nc.scalar)
            for k in range(KT2):
                nc.gpsimd.dma_start(w2_sb[:, k, :], x_like(w2_d)[k])

            qa_kt_tiles = [None] * nt
            ct_tiles = [None] * nt

            def fc2_y(jt):
                qa_kt = qa_kt_tiles[jt]
                c_t = ct_tiles[jt]
                for op_ in range(2):
                    pa = ps2.tile([128, 512], F32, tag="pa")
                    pb = ps2.tile([128, 512], F32, tag="pb")
                    for k in range(KT2):
                        nc.tensor.matmul(
                            pa, qa_kt[:, ts(k, 128)], w2_sb[:, k, ts(2 * op_, 512)],
                            start=(k == 0), stop=(k == KT2 - 1),
                        )
                        nc.tensor.matmul(
                            pb, qa_kt[:, ts(k, 128)], w2_sb[:, k, ts(2 * op_ + 1, 512)],
                            start=(k == 0), stop=(k == KT2 - 1),
                        )
                    for which, pp in ((0, pa), (1, pb)):
                        oc = (2 * op_ + which) * 512
                        yc = ycp.tile([128, 512], BF16)
                        nc.vector.scalar_tensor_tensor(
                            yc, pp, c_t, s2w_b[:, ds(oc, 512)], mul, mul
                        )
                        qeng = (nc.sync, nc.scalar, nc.gpsimd, nc.scalar)[2 * op_ + which]
                        qeng.dma_start(
                            x_like(y_d)[jt * 128 : (jt + 1) * 128, ds(oc, 512)], yc
                        )

            for it in range(nt):
                if it + 2 < nt:
                    load_tile_inputs(it + 2)
                qt_t = qt_tiles[it]
                rs_ap = qt_t[:, KT1 * 128 : KT1 * 128 + 4].bitcast(F32)
                r_ap = rs_ap[:, 0:1]
                sin_ap = rs_ap[:, 1:2]

                # ---- fc1 + per-group dequant/SwiGLU + group abs-max ----
                act_t = actp.tile([128, I], F32)
                mgs = []
                for gi, (off, cw) in enumerate(fc1_groups):
                    pg = ps1.tile([128, 512], F32, tag="pg")
                    pu = ps1.tile([128, 512], F32, tag="pu")
                    for k in range(KT1):
                        nc.tensor.matmul(
                            pg[:, :cw], qt_t[:, ts(k, 128)], w13_sb[:, k, ds(off, cw)],
                            start=(k == 0), stop=(k == KT1 - 1),
                        )
                        nc.tensor.matmul(
                            pu[:, :cw], qt_t[:, ts(k, 128)], w13_sb[:, k, ds(I + off, cw)],
                            start=(k == 0), stop=(k == KT1 - 1),
                        )
                    # act' = silu(g*s13g*s_in) * (u*s13u); true act = act' * s_in
                    g_sc = gp.tile([128, 512], F32)
                    nc.vector.scalar_tensor_tensor(
                        g_sc[:, :cw], pg[:, :cw], sin_ap, s13g_b[:, ds(off, cw)],
                        mul, mul,
                    )
                    nc.scalar.activation(
                        g_sc[:, :cw], g_sc[:, :cw],
                        mybir.ActivationFunctionType.Silu, bias=zero_b,
                    )
                    u_sc = up.tile([128, 512], F32)
                    nc.vector.tensor_tensor(
                        u_sc[:, :cw], pu[:, :cw], s13u_b[:, ds(off, cw)], mul
                    )
                    nc.vector.tensor_tensor(
                        act_t[:, ds(off, cw)], g_sc[:, :cw], u_sc[:, :cw], mul
                    )
                    mg = sp.tile([128, 1], F32, tag=f"mg{gi}")
                    nc.vector.tensor_reduce(
                        mg, act_t[:, ds(off, cw)], axis=mybir.AxisListType.X,
                        op=amax, apply_absolute_value=True,
                    )
                    mgs.append(mg)

                # ---- requant scale chain (vector; overlaps fc2(it-1) on PE) ----
                m2 = sp.tile([128, 1], F32, tag="m2")
                nc.vector.tensor_tensor(m2, mgs[0], mgs[1], amax)
                m2b = sp.tile([128, 1], F32, tag="m2b")
                nc.vector.tensor_tensor(m2b, m2, mgs[2], amax)
                mt2 = sp.tile([128, 1], F32, tag="mt2")
                nc.vector.tensor_tensor(mt2, m2b, sin_ap, mul)
                s_tr = sp.tile([128, 1], F32, tag="s_tr")
                nc.vector.tensor_scalar(s_tr, mt2, 1.0 / 127.0, 1e-8, mul, amax)
                inv_tr = sp.tile([128, 1], F32, tag="inv_tr")
                nc.vector.reciprocal(inv_tr, s_tr)
                sc_eff = sp.tile([128, 1], F32, tag="sc_eff")  # s_in / s_true
                nc.vector.tensor_tensor(sc_eff, sin_ap, inv_tr, mul)
                c_t = sp.tile([128, 1], F32, tag="c")  # r * s_true
                nc.vector.tensor_tensor(c_t, r_ap, s_tr, mul)
                ct_tiles[it] = c_t[:, 0:1]

                # ---- re-quantize act (round via +/- 1.5*2^23, RNE) ----
                # chunked so the first transposes can start before the tail
                # chunks convert (the PE meanwhile runs fc2(it-1))
                qa_sb = qasp.tile([128, I], BF16)
                for off, cw in fc1_groups:
                    t_c = tp.tile([128, 512], F32)
                    nc.vector.tensor_scalar(
                        t_c[:, :cw], act_t[:, ds(off, cw)], sc_eff[:, 0:1], TWO23,
                        mul, aadd,
                    )
                    nc.vector.tensor_scalar(
                        qa_sb[:, ds(off, cw)], t_c[:, :cw], -TWO23, None, aadd
                    )

                # ---- fc2 + combine for the PREVIOUS tile (keeps PE busy) ----
                if it > 0:
                    fc2_y(it - 1)

                # ---- transpose re-quantized act to contraction-major ----
                pst_t = pst.tile([128, KT2 * 128], BF16)
                for k in range(KT2):
                    nc.tensor.transpose(
                        pst_t[:, ts(k, 128)], qa_sb[:, ts(k, 128)], ident
                    )
                qa_kt = qaktp.tile([128, KT2 * 128], BF16)
                for off in range(0, KT2 * 128, 512):
                    cw = min(512, KT2 * 128 - off)
                    nc.scalar.copy(qa_kt[:, ds(off, cw)], pst_t[:, ds(off, cw)])
                qa_kt_tiles[it] = qa_kt

            fc2_y(nt - 1)

    nc.finalize()
    return nc


def kernel(hidden_states, gate_weight, w13_weight, w13_weight_scale,
           w2_weight, w2_weight_scale):
    x = np.ascontiguousarray(np.asarray(hidden_states, dtype=np.float32))
    gw = np.asarray(gate_weight, dtype=np.float32)
    w13 = np.asarray(w13_weight)
    s13 = np.ascontiguousarray(np.asarray(w13_weight_scale, dtype=np.float32))
    w2 = np.asarray(w2_weight)
    s2w = np.ascontiguousarray(np.asarray(w2_weight_scale, dtype=np.float32))

    # ---- host routing: fp32 gate, softmax, top-2, renormalize ----
    logits = (x @ gw.T).astype(np.float32)
    p = np.exp(logits - logits.max(axis=1, keepdims=True), dtype=np.float32)
    p = (p / p.sum(axis=1, keepdims=True)).astype(np.float32)
    topi = np.argsort(-p, axis=1, kind="stable")[:, :TOPK]  # ties -> lower index
    topv = np.take_along_axis(p, topi, axis=1).astype(np.float32)
    gates = (topv / topv.sum(axis=1, keepdims=True)).astype(np.float32)

    # ---- host dynamic per-token int8 quant (mirrors reference fp32 math) ----
    ax = np.abs(x).max(axis=1, keepdims=True).astype(np.float32)
    s_in = np.maximum(ax / np.float32(127.0), np.float32(1e-8)).astype(np.float32)
    q = np.clip(np.round(x / s_in), -127.0, 127.0).astype(np.float32)
    q_bf = q.astype(ml_dtypes.bfloat16)  # int8 values are exact in bf16

    idxs, rvals = [], []
    for e in range(E):
        sel = topi == e
        tok = np.nonzero(sel.any(axis=1))[0]
        r = (gates * sel)[tok].sum(axis=1).astype(np.float32)
        idxs.append(tok)
        rvals.append(r)

    cap = max(256, max(len(t) for t in idxs))
    C = ((cap + 127) // 128) * 128
    nt = C // 128

    if C not in _cache:
        _cache[C] = _build_program(C)
    nc = _cache[C]

    in_maps = []
    for e in range(E):
        n_e = len(idxs[e])
        qg = np.zeros((C, H), dtype=ml_dtypes.bfloat16)
        qg[:n_e] = q_bf[idxs[e]]
        rs = np.zeros((C, 2), dtype=np.float32)
        rs[:n_e, 0] = rvals[e]
        rs[:n_e, 1] = s_in[idxs[e], 0]
        # [tile, 128 h-in-k, k, 128 tok] blocks + bit-punned (r, s_in) tail
        qt_host = np.empty((nt, 128, KT1 * 128 + 4), dtype=ml_dtypes.bfloat16)
        qt_host[:, :, : KT1 * 128] = (
            qg.reshape(nt, 128, KT1, 128).transpose(0, 3, 2, 1)
            .reshape(nt, 128, KT1 * 128)
        )
        qt_host[:, :, KT1 * 128 :] = (
            rs.reshape(nt, 128, 2).view(np.uint16).view(ml_dtypes.bfloat16)
        )
        w13t = np.ascontiguousarray(w13[e].T).reshape(KT1, 128, 2 * I)
        w2t = np.ascontiguousarray(w2[e].T).reshape(KT2, 128, H)
        in_maps.append({
            "qt": qt_host,
            "w13t": w13t.astype(ml_dtypes.bfloat16),
            "w13i": w13t.astype(np.int8),
            "w2t": w2t.astype(ml_dtypes.bfloat16),
            "s13": np.ascontiguousarray(s13[e]),
            "s2w": np.ascontiguousarray(s2w[e]),
        })

    trace = bool(int(os.environ.get("MOE_TRACE", "0")))
    br = run_bass_kernel_spmd(nc, in_maps, list(range(E)), trace=trace)
    global LAST_EXEC_NS
    LAST_EXEC_NS = br.exec_time_ns
    res = br.results

    out = np.zeros((T, H), dtype=np.float32)
    for e in range(E):
        n_e = len(idxs[e])
        if n_e:
            out[idxs[e]] += np.asarray(res[e]["y"])[:n_e].astype(np.float32)
    return out


# revision 44
# speedup vs baseline: 1.0813x; 1.0382x over previous
"""Trainium2 Bass kernel for nn_IxformerQuantMoe (quantized top-2 MoE, E=8 experts).

Strategy (expert-parallel across 8 NeuronCores):
  - Host computes the fp32 gate (softmax + top-2 + renormalize), routes tokens
    to experts (padded to a common capacity C), and performs the per-token
    dynamic int8 quantization of the routed input, delivering it pre-transposed
    (contraction-major, int8 values exact in bf16).
  - Core e runs the quantized FFN for expert e, software-pipelined two tiles
    deep so the PE array never idles:
      PE stream per iteration: fc1(it) -> fc2(it-1) -> requant-transposes(it).
    The requant chain (dequant+SwiGLU, abs-max, rescale, re-quantize) for tile
    `it` executes on the vector/scalar engines concurrently with fc2(it-1).
    int8 x int8 products are computed exactly on the PE in bf16 (fp32 PSUM).
  - Host scatter-adds each expert's output rows into the final [T, H] output.
"""

import os
import sys

for _p in ("/opt/trn_rl_repo", "/root/.axon_site/_ro/trn_rl_repo"):
    if os.path.isdir(_p) and _p not in sys.path:
        sys.path.insert(0, _p)

import numpy as np
import ml_dtypes

import concourse.bass as bass
import concourse.bacc as bacc
import concourse.tile as tile
from concourse import mybir
from concourse.bass import ds, ts
from concourse.bass_utils import run_bass_kernel_spmd
from concourse.masks import make_identity

T, H, I, E, TOPK = 4096, 2048, 1408, 8, 2
KT1 = H // 128     # 16 k-tiles for fc1 contraction
KT2 = I // 128     # 11 k-tiles for fc2 contraction
TWO23 = 12582912.0  # 1.5*2^23: fp32 add/sub rounds to nearest integer (RNE)

F32 = mybir.dt.float32
BF16 = mybir.dt.bfloat16
MUL = None  # set at import below
_cache = {}
LAST_EXEC_NS = None


def x_like(handle):
    return handle[:]


def _bcast128(handle, off, n):
    """AP reading handle[off:off+n] replicated across 128 partitions."""
    ap = handle[:][ds(off, n)]
    return bass.AP(tensor=ap.tensor, offset=ap.offset, ap=[[0, 128]] + list(ap.ap))


def _build_program(C):
    nt = C // 128
    nc = bacc.Bacc(None, target_bir_lowering=False)
    mul = mybir.AluOpType.mult
    amax = mybir.AluOpType.max
    aadd = mybir.AluOpType.add

    # qt ships as int8 and expands to bf16 inside an SWDGE casting DMA; the
    # per-token fp32 (r, s_in) pairs for ALL tiles ride one prologue DMA.
    qt_d = nc.declare_dram_parameter("qt", [nt, 128, KT1 * 128], mybir.dt.int8,
                                     isOutput=False)
    rsall_d = nc.declare_dram_parameter("rsall", [128, nt, 2], F32, isOutput=False)
    w13_d = nc.declare_dram_parameter("w13t", [KT1, 128, 2 * I], BF16, isOutput=False)
    w13i_d = nc.declare_dram_parameter(
        "w13i", [KT1, 128, 2 * I], mybir.dt.int8, isOutput=False
    )
    w2_d = nc.declare_dram_parameter("w2t", [KT2, 128, H], BF16, isOutput=False)
    s13_d = nc.declare_dram_parameter("s13", [2 * I], F32, isOutput=False)
    s2w_d = nc.declare_dram_parameter("s2w", [H], F32, isOutput=False)
    y_d = nc.declare_dram_parameter("y", [C, H], BF16, isOutput=True)

    fc1_groups = [(0, 512), (512, 512), (1024, 384)]

    with tile.TileContext(nc) as tc:
        with (
            tc.tile_pool(name="singles", bufs=1) as singles,
            tc.tile_pool(name="qtp", bufs=3) as qtp,
            tc.tile_pool(name="gp", bufs=2) as gp,
            tc.tile_pool(name="up", bufs=2) as up,
            tc.tile_pool(name="actp", bufs=1) as actp,
            tc.tile_pool(name="tp", bufs=2) as tp,
            tc.tile_pool(name="qasp", bufs=1) as qasp,
            tc.tile_pool(name="qaktp", bufs=1) as qaktp,
            tc.tile_pool(name="ycp", bufs=3) as ycp,
            tc.tile_pool(name="sp", bufs=2) as sp,
            tc.tile_pool(name="ps1", bufs=2, space="PSUM") as ps1,
            tc.tile_pool(name="ps2", bufs=2, space="PSUM") as ps2,
            tc.tile_pool(name="pst", bufs=1, space="PSUM") as pst,
        ):
            # ---- resident weights / scales; w13 split over 4 DMA queues ----
            w13_sb = singles.tile([128, KT1, 2 * I], BF16)
            w2_sb = singles.tile([128, KT2, H], BF16)
            s13g_b = singles.tile([128, I], F32)
            s13u_b = singles.tile([128, I], F32)
            s2w_b = singles.tile([128, H], F32)
            ident = singles.tile([128, 128], BF16)
            zero_b = singles.tile([128, 1], F32)

            qt_tiles = [None] * nt

            def load_tile_inputs(it):
                # int8 -> bf16 conversion happens inside the gpsimd SWDGE
                q = qtp.tile([128, KT1 * 128], BF16, name=f"qtt{it}", tag="qt")
                nc.gpsimd.dma_start(q, x_like(qt_d)[it])
                qt_tiles[it] = q

            # Lead-in schedule: all DMA rings share one pool of 16 DMA
            # engines (~320GB/s aggregate), so the w13 stream is byte-bound.
            # qt(0) heads the sync ring so the first matmul isn't queued
            # behind it on the ring that carries w13 k=0; gpsimd's share of
            # w13 rides as int8 with an SWDGE casting DMA (half the read
            # bytes); scale broadcasts go mid-stream; w2 trails on gpsimd.
            load_tile_inputs(0)
            rs_all = singles.tile([128, nt, 2], F32)
            nc.sync.dma_start(rs_all, x_like(rsall_d))
            make_identity(nc, ident)
            nc.vector.memset(zero_b, 0.0)
            # every third w13 k-chunk rides the gpsimd SWDGE as an int8
            # casting DMA (half the read bytes; more than that overloads the
            # SWDGE); the rest alternate scalar/sync as bf16; broadcasts
            # mid-stream; w2 trails on gpsimd.
            rings = (nc.scalar, nc.gpsimd, nc.sync)
            for k in range(KT1):
                if k % 3 == 1:
                    nc.gpsimd.dma_start(w13_sb[:, k, :], x_like(w13i_d)[k])
                else:
                    rings[k % 3].dma_start(w13_sb[:, k, :], x_like(w13_d)[k])
                if k == 11:
                    nc.sync.dma_start(s13g_b, _bcast128(s13_d, 0, I))
                    nc.scalar.dma_start(s13u_b, _bcast128(s13_d, I, I))
                    nc.gpsimd.dma_start(s2w_b, _bcast128(s2w_d, 0, H))
            load_tile_inputs(1)
            for k in range(KT2):
                nc.gpsimd.dma_start(w2_sb[:, k, :], x_like(w2_d)[k])

            qa_kt_tiles = [None] * nt
            ct_tiles = [None] * nt

            def fc2_y(jt):
                qa_kt = qa_kt_tiles[jt]
                c_t = ct_tiles[jt]
                for h in range(4):
                    pp = ps2.tile([128, 512], F32, tag="pa")
                    for k in range(KT2):
                        nc.tensor.matmul(
                            pp, qa_kt[:, ts(k, 128)], w2_sb[:, k, ts(h, 512)],
                            start=(k == 0), stop=(k == KT2 - 1),
                        )
                    oc = h * 512
                    yc = ycp.tile([128, 512], BF16)
                    nc.vector.scalar_tensor_tensor(
                        yc, pp, c_t, s2w_b[:, ds(oc, 512)], mul, mul
                    )
                    qeng = (nc.sync, nc.scalar, nc.gpsimd, nc.scalar)[h]
                    qeng.dma_start(
                        x_like(y_d)[jt * 128 : (jt + 1) * 128, ds(oc, 512)], yc
                    )

            for it in range(nt):
                if it + 2 < nt:
                    load_tile_inputs(it + 2)
                qt_t = qt_tiles[it]
                r_ap = rs_all[:, it, 0:1]
                sin_ap = rs_all[:, it, 1:2]

                # ---- fc1 + per-group dequant/SwiGLU + group abs-max ----
                act_t = actp.tile([128, I], F32)
                mgs = []
                for gi, (off, cw) in enumerate(fc1_groups):
                    pg = ps1.tile([128, 512], F32, tag="pg")
                    pu = ps1.tile([128, 512], F32, tag="pu")
                    for k in range(KT1):
                        nc.tensor.matmul(
                            pg[:, :cw], qt_t[:, ts(k, 128)], w13_sb[:, k, ds(off, cw)],
                            start=(k == 0), stop=(k == KT1 - 1),
                        )
                        nc.tensor.matmul(
                            pu[:, :cw], qt_t[:, ts(k, 128)], w13_sb[:, k, ds(I + off, cw)],
                            start=(k == 0), stop=(k == KT1 - 1),
                        )
                    # act' = silu(g*s13g*s_in) * (u*s13u); true act = act' * s_in
                    g_sc = gp.tile([128, 512], F32)
                    nc.vector.scalar_tensor_tensor(
                        g_sc[:, :cw], pg[:, :cw], sin_ap, s13g_b[:, ds(off, cw)],
                        mul, mul,
                    )
                    nc.scalar.activation(
                        g_sc[:, :cw], g_sc[:, :cw],
                        mybir.ActivationFunctionType.Silu, bias=zero_b,
                    )
                    u_sc = up.tile([128, 512], F32)
                    nc.vector.tensor_tensor(
                        u_sc[:, :cw], pu[:, :cw], s13u_b[:, ds(off, cw)], mul
                    )
                    nc.vector.tensor_tensor(
                        act_t[:, ds(off, cw)], g_sc[:, :cw], u_sc[:, :cw], mul
                    )
                    mg = sp.tile([128, 1], F32, tag=f"mg{gi}")
                    nc.vector.tensor_reduce(
                        mg, act_t[:, ds(off, cw)], axis=mybir.AxisListType.X,
                        op=amax, apply_absolute_value=True,
                    )
                    mgs.append(mg)

                # ---- requant scale chain (vector; overlaps fc2(it-1) on PE) ----
                m2 = sp.tile([128, 1], F32, tag="m2")
                nc.vector.tensor_tensor(m2, mgs[0], mgs[1], amax)
                m2b = sp.tile([128, 1], F32, tag="m2b")
                nc.vector.tensor_tensor(m2b, m2, mgs[2], amax)
                mt2 = sp.tile([128, 1], F32, tag="mt2")
                nc.vector.tensor_tensor(mt2, m2b, sin_ap, mul)
                s_tr = sp.tile([128, 1], F32, tag="s_tr")
                nc.vector.tensor_scalar(s_tr, mt2, 1.0 / 127.0, 1e-8, mul, amax)
                inv_tr = sp.tile([128, 1], F32, tag="inv_tr")
                nc.vector.reciprocal(inv_tr, s_tr)
                sc_eff = sp.tile([128, 1], F32, tag="sc_eff")  # s_in / s_true
                nc.vector.tensor_tensor(sc_eff, sin_ap, inv_tr, mul)
                c_t = sp.tile([128, 1], F32, tag="c")  # r * s_true
                nc.vector.tensor_tensor(c_t, r_ap, s_tr, mul)
                ct_tiles[it] = c_t[:, 0:1]

                # ---- re-quantize act (round via +/- 1.5*2^23, RNE) ----
                # chunked so the first transposes can start before the tail
                # chunks convert (the PE meanwhile runs fc2(it-1))
                qa_sb = qasp.tile([128, I], BF16)
                for off, cw in fc1_groups:
                    t_c = tp.tile([128, 512], F32)
                    nc.vector.tensor_scalar(
                        t_c[:, :cw], act_t[:, ds(off, cw)], sc_eff[:, 0:1], TWO23,
                        mul, aadd,
                    )
                    nc.vector.tensor_scalar(
                        qa_sb[:, ds(off, cw)], t_c[:, :cw], -TWO23, None, aadd
                    )

                # ---- fc2 + combine for the PREVIOUS tile (keeps PE busy) ----
                if it > 0:
                    fc2_y(it - 1)

                # ---- transpose re-quantized act to contraction-major ----
                pst_t = pst.tile([128, KT2 * 128], BF16)
                for k in range(KT2):
                    nc.tensor.transpose(
                        pst_t[:, ts(k, 128)], qa_sb[:, ts(k, 128)], ident
                    )
                qa_kt = qaktp.tile([128, KT2 * 128], BF16)
                for off in range(0, KT2 * 128, 512):
                    cw = min(512, KT2 * 128 - off)
                    nc.scalar.copy(qa_kt[:, ds(off, cw)], pst_t[:, ds(off, cw)])
                qa_kt_tiles[it] = qa_kt

            fc2_y(nt - 1)

    nc.finalize()
    return nc


def kernel(hidden_states, gate_weight, w13_weight, w13_weight_scale,
           w2_weight, w2_weight_scale):
    x = np.ascontiguousarray(np.asarray(hidden_states, dtype=np.float32))
    gw = np.asarray(gate_weight, dtype=np.float32)
    w13 = np.asarray(w13_weight)
    s13 = np.ascontiguousarray(np.asarray(w13_weight_scale, dtype=np.float32))
    w2 = np.asarray(w2_weight)
    s2w = np.ascontiguousarray(np.asarray(w2_weight_scale, dtype=np.float32))

    # ---- host routing: fp32 gate, softmax, top-2, renormalize ----
    logits = (x @ gw.T).astype(np.float32)
    p = np.exp(logits - logits.max(axis=1, keepdims=True), dtype=np.float32)
    p = (p / p.sum(axis=1, keepdims=True)).astype(np.float32)
    topi = np.argsort(-p, axis=1, kind="stable")[:, :TOPK]  # ties -> lower index
    topv = np.take_along_axis(p, topi, axis=1).astype(np.float32)
    gates = (topv / topv.sum(axis=1, keepdims=True)).astype(np.float32)

    # ---- host dynamic per-token int8 quant (mirrors reference fp32 math) ----
    ax = np.abs(x).max(axis=1, keepdims=True).astype(np.float32)
    s_in = np.maximum(ax / np.float32(127.0), np.float32(1e-8)).astype(np.float32)
    q = np.clip(np.round(x / s_in), -127.0, 127.0).astype(np.float32)
    q_bf = q.astype(ml_dtypes.bfloat16)  # int8 values are exact in bf16

    idxs, rvals = [], []
    for e in range(E):
        sel = topi == e
        tok = np.nonzero(sel.any(axis=1))[0]
        r = (gates * sel)[tok].sum(axis=1).astype(np.float32)
        idxs.append(tok)
        rvals.append(r)

    cap = max(256, max(len(t) for t in idxs))
    C = ((cap + 127) // 128) * 128
    nt = C // 128

    if C not in _cache:
        _cache[C] = _build_program(C)
    nc = _cache[C]

    in_maps = []
    for e in range(E):
        n_e = len(idxs[e])
        qg = np.zeros((C, H), dtype=ml_dtypes.bfloat16)
        qg[:n_e] = q_bf[idxs[e]]
        rs = np.zeros((C, 2), dtype=np.float32)
        rs[:n_e, 0] = rvals[e]
        rs[:n_e, 1] = s_in[idxs[e], 0]
        # [tile, 128 h-in-k, k, 128 tok] int8 blocks (cast to bf16 in-DMA)
        qt_host = np.ascontiguousarray(
            qg.astype(np.int8).reshape(nt, 128, KT1, 128).transpose(0, 3, 2, 1)
            .reshape(nt, 128, KT1 * 128)
        )
        w13t = np.ascontiguousarray(w13[e].T).reshape(KT1, 128, 2 * I)
        w2t = np.ascontiguousarray(w2[e].T).reshape(KT2, 128, H)
        in_maps.append({
            "qt": qt_host,
            "rsall": np.ascontiguousarray(rs.reshape(nt, 128, 2).transpose(1, 0, 2)),
            "w13t": w13t.astype(ml_dtypes.bfloat16),
            "w13i": w13t.astype(np.int8),
            "w2t": w2t.astype(ml_dtypes.bfloat16),
            "s13": np.ascontiguousarray(s13[e]),
            "s2w": np.ascontiguousarray(s2w[e]),
        })

    trace = bool(int(os.environ.get("MOE_TRACE", "0")))
    br = run_bass_kernel_spmd(nc, in_maps, list(range(E)), trace=trace)
    global LAST_EXEC_NS
    LAST_EXEC_NS = br.exec_time_ns
    res = br.results

    out = np.zeros((T, H), dtype=np.float32)
    for e in range(E):
        n_e = len(idxs[e])
        if n_e:
            out[idxs[e]] += np.asarray(res[e]["y"])[:n_e].astype(np.float32)
    return out
